# revision 31
# baseline (speedup 1.0000x reference)
"""Trainium2 Bass kernel for NeuronInvariantDeepSetLayer (segment_reduce).

kernel(**inputs) takes FULL unsharded inputs (as in reference.setup_inputs())
and returns the full [4096, 1] float32 output.

Strategy: data-parallel over 8 NeuronCores. Segments are split 512/core
(idx is sorted, so each core's rows are a contiguous slice of x). Rows are
host-padded so that each 128-segment block starts exactly at a 128-row tile
boundary -> every core runs the IDENTICAL instruction stream (pure SPMD),
only the data differs.

Two structural optimizations over the per-row mm1/mm2 formulation:
  1. x is transposed and cast to bf16 on the HOST. The device DMA reads
     xT [feat, rows] directly (2KB contiguous runs/partition), halving HBM
     traffic and eliminating all PE transposes of x.
  2. mm2 is deferred past the segment sum and folded into rho on the host:
       seg_sum(relu(x@W1+b1) @ W2 + b2) @ rho_w1
         = seg_sum(relu(x@W1+b1)) @ (W2 @ rho_w1) + counts * (b2 @ rho_w1)
     so the device only computes pseg = seg_sum(relu(x@W1)) [segs, 192] and
     a tiny per-block rho with E = W2 @ rho_w1 [192, 6]. The 400k-row mm2
     disappears entirely.

Per core device pipeline (fp8 mm1 / f32 psum accumulate / f32 rho):
  - DMA: xT chunks fp8e4m3 HBM -> SBUF [128 feat, 6 kc, 1024 rows]
    (x quantized to fp8 on host; W1 pre-scaled x16 into fp8 normal range,
    the 1/16 is folded exactly into E = W2 @ rho_w1 / 16)
  - mm1 per 128-row tile: out h1 [rows, 192] = sum_kc xT_kc.T @ W1_kc (PE,
    fp8 with FWL -- same stream rate as bf16 but half the DMA bytes)
  - ACT relu psum -> SBUF bf16 [rows, 192]
  - sel = is_equal(idx_local, iota) one-hot [128 rows, 128 segs] (DVE)
  - seg reduce: matmul(pseg += sel.T @ h1) accumulated in PSUM over ~tblk
    tiles -> pseg [128 segs, 192]; emission is pipelined 2 tiles behind
    mm1 so the relu/sel semaphores are already satisfied when PE gets there
  - rho (tiny, f32): transpose pseg, 2 matmuls + relu -> out [128] per block
"""

import sys

sys.path.insert(0, "/opt/trn_rl_repo")

import numpy as np
import ml_dtypes

N = 400000
B = 4096
DIN = 768
DHID = 192
NCORES = 8
SPC = B // NCORES  # segments per core = 512
SBLK = 128  # segments per seg-block (psum accumulator width)
NBLK = SPC // SBLK  # 4 seg-blocks per core
P = 128
KC1 = DIN // P  # 6 k-chunks for mm1
CH = 1024  # rows per DMA chunk (8 row-tiles)

f32 = np.float32
bf16 = ml_dtypes.bfloat16
fp8 = ml_dtypes.float8_e4m3
W1_SCALE = 16.0  # pre-scale W1 into fp8 normal range; 1/16 folded into E


def _prep(x, idx):
    """Host-side sharding. Returns per-core transposed bf16 shards + params."""
    if np.any(np.diff(idx) < 0):  # defensive: spec says idx is sorted
        order = np.argsort(idx, kind="stable")
        x, idx = x[order], idx[order]
    counts = np.bincount(idx, minlength=B)
    assert counts.sum() == x.shape[0]
    bounds = np.concatenate([[0], np.cumsum(counts)]).astype(np.int64)
    blk_rows = counts.reshape(NCORES * NBLK, SBLK).sum(1)
    tblk = int(np.ceil(blk_rows.max() / P))
    tblk = ((tblk + 1) // 2) * 2  # even -> NP % 1024 == 0
    NP = NBLK * tblk * P
    nchunks = NP // CH
    xb = x.astype(fp8)
    # chunk-tiled layout: xts[c, ch, p, kc, n] = x_pad[ch*CH + n, kc*128 + p]
    # -> every chunk DMA is 128 descriptors of KC1*CH contiguous bytes
    xts = np.zeros((NCORES, NP, DIN), fp8)
    ixs = np.full((NCORES, NP), 1.0e9, f32)
    for c in range(NCORES):
        for blk in range(NBLK):
            s0 = c * SPC + blk * SBLK
            r0, r1 = int(bounds[s0]), int(bounds[s0 + SBLK])
            d0 = blk * tblk * P
            xts[c, d0 : d0 + (r1 - r0)] = xb[r0:r1]
            ixs[c, d0 : d0 + (r1 - r0)] = (idx[r0:r1] - c * SPC).astype(f32)
    xts = np.ascontiguousarray(
        xts.reshape(NCORES, nchunks, CH, KC1, P).transpose(0, 1, 4, 3, 2)
    )
    # pre-arrange idx so each partition's DMA read is contiguous:
    # ixs_arr[c, ch, p, n] = ixs[c, ch*CH + n*P + p]
    ixs_arr = np.ascontiguousarray(
        ixs.reshape(NCORES, nchunks, CH // P, P).transpose(0, 1, 3, 2)
    )
    return xts, ixs_arr, tblk, counts


def _build(tblk, phi_w1, phi_b1, phi_w2, phi_b2, rho_w1, rho_b1, rho_w2, rho_b2):
    import concourse.bacc as bacc
    import concourse.mybir as mybir
    import concourse.tile as tile

    BF = mybir.dt.bfloat16
    F32 = mybir.dt.float32
    FP8 = mybir.dt.float8e4
    Relu = mybir.ActivationFunctionType.Relu
    Copy = mybir.ActivationFunctionType.Copy

    has_b1 = bool(np.any(phi_b1 != 0))
    has_b2 = bool(np.any(phi_b2 != 0))
    has_rb1 = bool(np.any(rho_b1 != 0))
    has_rb2 = bool(np.any(rho_b2 != 0))

    # ---- packed constants (inlined into the NEFF) ----
    # w1k[p, kc, h] = W1[kc*128 + p, h] * 16 (fp8 normal range)
    w1k = np.ascontiguousarray(
        (phi_w1 * W1_SCALE).reshape(KC1, P, DHID).transpose(1, 0, 2)
    ).astype(fp8)
    # E = W2 @ rho_w1 / 16  [192, 6] (mm2 + the 1/16 folded into rho)
    E = (phi_w2.astype(np.float64) @ rho_w1.astype(np.float64)).astype(f32) / W1_SCALE
    rw1k = np.ascontiguousarray(E.reshape(2, 96, 6).transpose(1, 0, 2)).astype(f32)
    rw2k = np.ascontiguousarray(rho_w2).astype(f32)  # [6, 1]
    idn32 = np.eye(P, dtype=f32)
    jmat = np.ascontiguousarray(
        np.broadcast_to(
            (np.arange(NBLK)[:, None] * SBLK + np.arange(SBLK)[None, :]).astype(f32),
            (P, NBLK, SBLK),
        )
    )
    rb1k = np.ascontiguousarray(rho_b1.reshape(6, 1)).astype(f32)
    rb2k = np.ascontiguousarray(rho_b2.reshape(1, 1)).astype(f32)
    onesk = np.ones((1, P), bf16)
    b1k = np.ascontiguousarray(phi_b1.reshape(1, DHID) * W1_SCALE).astype(bf16)
    # beta = b2 @ rho_w1 [6]; pseg misses counts*b2, corrected in rho preact
    beta = (phi_b2.astype(np.float64) @ rho_w1.astype(np.float64)).astype(f32)
    betak = np.ascontiguousarray(beta.reshape(1, 6)).astype(f32)

    NP = NBLK * tblk * P
    nchunks = NP // CH

    nc = bacc.Bacc(None, target_bir_lowering=False)
    xt_in = nc.dram_tensor("xt_shard", [NP // CH, P, KC1, CH], FP8, kind="ExternalInput")
    ix_in = nc.dram_tensor("idxlf", [nchunks, P, CH // P], F32, kind="ExternalInput")
    cnt_in = (
        nc.dram_tensor("cnts", [1, NBLK, SBLK], F32, kind="ExternalInput")
        if has_b2
        else None
    )
    out_d = nc.dram_tensor("out_shard", [SPC], F32, kind="ExternalOutput")

    w1d = nc.inline_tensor(w1k, "w1k")
    rw1d = nc.inline_tensor(rw1k, "rw1k")
    rw2d = nc.inline_tensor(rw2k, "rw2k")
    idn32d = nc.inline_tensor(idn32, "idn32")
    jmatd = nc.inline_tensor(jmat, "jmat")
    rb1d = nc.inline_tensor(rb1k, "rb1k") if has_rb1 else None
    rb2d = nc.inline_tensor(rb2k, "rb2k") if has_rb2 else None
    onesd = nc.inline_tensor(onesk, "onesk") if has_b1 else None
    b1d = nc.inline_tensor(b1k, "b1k") if has_b1 else None
    betad = nc.inline_tensor(betak, "betak") if has_b2 else None

    with tile.TileContext(nc) as tc:
        with (
            tc.tile_pool(name="consts", bufs=1) as cpool,
            tc.tile_pool(name="xb", bufs=6) as xpool,
            tc.tile_pool(name="ixb", bufs=6) as ixpool,
            tc.tile_pool(name="h1b", bufs=6) as h1pool,
            tc.tile_pool(name="selb", bufs=6) as selpool,
            tc.tile_pool(name="rho", bufs=2) as rhopool,
            tc.tile_pool(name="ph1", bufs=4, space="PSUM") as ph1,
            tc.tile_pool(name="pxt", bufs=2, space="PSUM") as pxt,
            tc.tile_pool(name="pseg", bufs=2, space="PSUM") as pseg,
        ):
            # w1s is the only const mm1 needs -- load it first so PE can
            # start as soon as the first x piece lands; everything else
            # (sel iota, rho consts) is loaded after the first x DMAs.
            w1s = cpool.tile_from(w1d[:], name="w1s")
            late = {}

            def load_late_consts():
                late["js"] = cpool.tile_from(jmatd[:], name="js")
                late["rw1s"] = cpool.tile_from(rw1d[:], name="rw1s")
                late["rw2s"] = cpool.tile_from(rw2d[:], name="rw2s")
                late["idn32s"] = cpool.tile_from(idn32d[:], name="idn32s")
                late["rb1s"] = cpool.tile_from(rb1d[:], name="rb1s") if has_rb1 else None
                late["rb2s"] = cpool.tile_from(rb2d[:], name="rb2s") if has_rb2 else None
                late["oness"] = cpool.tile_from(onesd[:], name="oness") if has_b1 else None
                late["b1s"] = cpool.tile_from(b1d[:], name="b1s") if has_b1 else None
                late["betas"] = cpool.tile_from(betad[:], name="betas") if has_b2 else None
                late["cnts"] = cpool.tile_from(cnt_in[:], name="cnts") if has_b2 else None

            pseg_tiles = {}
            pend_seg = []  # [(selb, h1s, blk, pos)] 2-deep PE emission queue
            pend_rho = None  # (blk, pseg_tile)

            def emit_seg(selb_t, h1s_t, blk, pos):
                nc.tensor.matmul(
                    out=pseg_tiles[blk][:],
                    lhsT=selb_t[:],
                    rhs=h1s_t[:],
                    start=(pos == 0),
                    stop=(pos == tblk - 1),
                )
                if pos == tblk - 1:
                    return (blk, pseg_tiles.pop(blk))
                return None

            def emit_rho(blk, pseg_t):
                # pseg [128 segs, 192] f32 psum -> out[blk*128:(blk+1)*128]
                xsb = rhopool.tile([P, DHID], F32, tag="xsb")
                nc.scalar.copy(out=xsb[:], in_=pseg_t[:])
                pxsT = pxt.tile([96, 2, P], F32, tag="xt")
                for m2 in range(2):
                    nc.tensor.transpose(
                        out=pxsT[:, m2, :],
                        in_=xsb[:, m2 * 96 : (m2 + 1) * 96],
                        identity=late["idn32s"][:],
                    )
                xsTb = rhopool.tile([96, 2, P], F32, tag="xsTb")
                nc.vector.tensor_copy(out=xsTb[:], in_=pxsT[:])
                prt = pxt.tile([6, P], F32, tag="xt")
                for m2 in range(2):
                    nc.tensor.matmul(
                        out=prt[:],
                        lhsT=late["rw1s"][:, m2, :],
                        rhs=xsTb[:, m2, :],
                        start=(m2 == 0),
                        stop=(m2 == 1 and not has_b2),
                    )
                if has_b2:
                    nc.tensor.matmul(
                        out=prt[:], lhsT=late["betas"][:], rhs=late["cnts"][:, blk, :],
                        start=False, stop=True,
                    )
                rtb = rhopool.tile([6, P], F32, tag="rtb")
                if has_rb1:
                    nc.scalar.activation(
                        out=rtb[:], in_=prt[:], func=Relu, bias=late["rb1s"][:]
                    )
                else:
                    nc.scalar.activation(out=rtb[:], in_=prt[:], func=Relu)
                pot = pxt.tile([1, P], F32, tag="xt")
                nc.tensor.matmul(out=pot[:], lhsT=late["rw2s"][:], rhs=rtb[:], start=True, stop=True)
                ob = rhopool.tile([1, P], F32, tag="ob")
                if has_rb2:
                    nc.scalar.activation(out=ob[:], in_=pot[:], func=Copy, bias=late["rb2s"][:])
                else:
                    nc.scalar.copy(out=ob[:], in_=pot[:])
                nc.sync.dma_start(out=out_d[blk * SBLK : (blk + 1) * SBLK], in_=ob[:])

            state = {"pend_seg": pend_seg, "pend_rho": pend_rho}

            def do_tile(t, i, xt_at, ixb, ph1_pre=None):
                blk = t // tblk
                pos = t % tblk
                # --- mm1: h1 [rows, 192] = x_tile @ W1 ---
                if ph1_pre is not None:
                    ph1t, kc0 = ph1_pre
                else:
                    ph1t = ph1.tile([P, DHID], F32, tag="h1", name=f"ph1_{t}")
                    kc0 = 0
                for kc in range(kc0, KC1):
                    nc.tensor.matmul(
                        out=ph1t[:],
                        lhsT=xt_at(i, kc),
                        rhs=w1s[:, kc, :],
                        start=(kc == 0),
                        stop=(kc == KC1 - 1 and not has_b1),
                    )
                if has_b1:
                    nc.tensor.matmul(
                        out=ph1t[:], lhsT=late["oness"][:], rhs=late["b1s"][:],
                        start=False, stop=True,
                    )
                # pipelined tails from 2 tiles ago (relu/sel sems are
                # already satisfied by the time PE reaches the seg matmul)
                if state["pend_rho"] is not None:
                    emit_rho(*state["pend_rho"])
                    state["pend_rho"] = None
                if len(state["pend_seg"]) >= 2:
                    state["pend_rho"] = emit_seg(*state["pend_seg"].pop(0))
                # --- relu + one-hot sel for this tile ---
                h1s = h1pool.tile([P, DHID], BF, tag="h1s", name=f"h1s_{t}")
                nc.scalar.activation(out=h1s[:], in_=ph1t[:], func=Relu)
                selb = selpool.tile([P, P], BF, tag="selb", name=f"sel_{t}")
                nc.vector.tensor_tensor(
                    out=selb[:],
                    in0=ixb[:, i : i + 1].to_broadcast([P, P]),
                    in1=late["js"][:, blk, :],
                    op=mybir.AluOpType.is_equal,
                )
                if pos == 0:
                    pseg_tiles[blk] = pseg.tile(
                        [P, DHID], F32, tag="seg", name=f"pseg_{blk}"
                    )
                state["pend_seg"].append((selb, h1s, blk, pos))

            # chunk 0: two kc-group pieces (128 descriptors of 3KB each --
            # cheap to generate AND transfer). mm1 for tiles 0-3 is split
            # into two passes (kc0-2 then kc3-5) so PE starts on piece A
            # while piece B is still in flight.
            xq_a = xpool.tile([P, 3, CH], FP8, tag="xb0a", name="xb0a", bufs=1)
            nc.gpsimd.dma_start(out=xq_a[:], in_=xt_in[0, :, 0:3, :])
            ixb0 = ixpool.tile([P, CH // P], F32, tag="ixb", name="ixb0")
            nc.sync.dma_start(out=ixb0[:], in_=ix_in[0])

            def xt_at0(i, kc):
                tt = xq_a if kc < 3 else xq_b
                return tt[:, kc % 3, i * P : (i + 1) * P]

            # warm pass emitted BEFORE piece B's dma_start: the DMA-queue
            # semaphore threshold for these matmuls then covers piece A only
            NWARM = 4  # tiles 0..3: kc0-2 first (piece A), then kc3-5
            ph1_warm = []
            for i in range(NWARM):
                ph1t = ph1.tile([P, DHID], F32, tag="h1", name=f"ph1_{i}")
                for kc in range(3):
                    nc.tensor.matmul(
                        out=ph1t[:],
                        lhsT=xt_at0(i, kc),
                        rhs=w1s[:, kc, :],
                        start=(kc == 0),
                        stop=False,
                    )
                ph1_warm.append(ph1t)
            xq_b = xpool.tile([P, 3, CH], FP8, tag="xb0b", name="xb0b", bufs=1)
            nc.gpsimd.dma_start(out=xq_b[:], in_=xt_in[0, :, 3:6, :])
            load_late_consts()
            for i in range(NWARM):
                do_tile(i, i, xt_at0, ixb0, ph1_pre=(ph1_warm[i], 3))
            for i in range(NWARM, CH // P):
                do_tile(i, i, xt_at0, ixb0)

            for ch in range(1, nchunks):
                xtb = xpool.tile([P, KC1, CH], FP8, tag="xtb", name=f"xtb_{ch}")
                nc.gpsimd.dma_start(out=xtb[:], in_=xt_in[ch])

                def xt_at(i, kc, _xtb=xtb):
                    return _xtb[:, kc, i * P : (i + 1) * P]

                ixb = ixpool.tile([P, CH // P], F32, tag="ixb", name=f"ixb_{ch}")
                nc.sync.dma_start(out=ixb[:], in_=ix_in[ch])
                for i in range(CH // P):
                    do_tile(ch * (CH // P) + i, i, xt_at, ixb)
            pend_seg = state["pend_seg"]
            pend_rho = state["pend_rho"]
            # drain
            while pend_seg:
                if pend_rho is not None:
                    emit_rho(*pend_rho)
                    pend_rho = None
                pend_rho = emit_seg(*pend_seg.pop(0))
            if pend_rho is not None:
                emit_rho(*pend_rho)
                pend_rho = None

    nc.compile()
    return nc


_CACHE = {}


def _get_nc(tblk, weights):
    key = (tblk, tuple(hash(w.tobytes()) for w in weights))
    if key not in _CACHE:
        _CACHE[key] = _build(tblk, *weights)
    return _CACHE[key]


def _run(inputs, trace=False):
    from concourse.bass_utils import run_bass_kernel_spmd

    inp = {k: np.asarray(v) for k, v in inputs.items()}
    x = inp["x"].astype(f32, copy=False)
    idx = inp["idx"].astype(np.int32, copy=False)
    weights = tuple(
        inp[k].astype(f32, copy=False)
        for k in ("phi_w1", "phi_b1", "phi_w2", "phi_b2", "rho_w1", "rho_b1", "rho_w2", "rho_b2")
    )
    xts, ixs, tblk, counts = _prep(x, idx)
    nc = _get_nc(tblk, weights)
    has_b2 = bool(np.any(weights[3] != 0))
    in_maps = []
    for c in range(NCORES):
        m = {"xt_shard": xts[c], "idxlf": ixs[c]}
        if has_b2:
            m["cnts"] = np.ascontiguousarray(
                counts[c * SPC : (c + 1) * SPC].reshape(1, NBLK, SBLK)
            ).astype(f32)
        in_maps.append(m)
    res = run_bass_kernel_spmd(nc, in_maps, core_ids=list(range(NCORES)), trace=trace)
    out = np.concatenate([res.results[c]["out_shard"] for c in range(NCORES)])
    out = out.reshape(B, 1).astype(f32)
    return out, res


def kernel(**inputs) -> np.ndarray:
    return _run(inputs, trace=False)[0]


if __name__ == "__main__":
    # quick self-test against numpy
    rng = np.random.default_rng(0)
    x = rng.standard_normal((N, DIN)).astype(f32)
    idx = np.sort(rng.integers(0, B, N).astype(np.int32))
    w1 = (rng.standard_normal((DIN, DHID)) / np.sqrt(DIN)).astype(f32)
    w2 = (rng.standard_normal((DHID, DHID)) / np.sqrt(DHID)).astype(f32)
    r1 = (rng.standard_normal((DHID, 6)) / np.sqrt(DHID)).astype(f32)
    r2 = (rng.standard_normal((6, 1)) / np.sqrt(6)).astype(f32)
    inputs = dict(
        x=x, idx=idx,
        phi_w1=w1, phi_b1=np.zeros(DHID, f32), phi_w2=w2, phi_b2=np.zeros(DHID, f32),
        rho_w1=r1, rho_b1=np.zeros(6, f32), rho_w2=r2, rho_b2=np.zeros(1, f32),
    )
    out = kernel(**inputs)
    h = np.maximum(x @ w1, 0.0) @ w2
    xsum = np.zeros((B, DHID), f32)
    np.add.at(xsum, idx, h)
    exp = np.maximum(xsum @ r1, 0.0) @ r2
    rel = np.linalg.norm(out - exp) / np.linalg.norm(exp)
    print("self-test rel err:", rel)


# revision 40
# speedup vs baseline: 1.0009x; 1.0009x over previous
"""Trainium2 Bass kernel for NeuronInvariantDeepSetLayer (segment_reduce).

kernel(**inputs) takes FULL unsharded inputs (as in reference.setup_inputs())
and returns the full [4096, 1] float32 output.

Strategy: data-parallel over 8 NeuronCores. Segments are split 512/core
(idx is sorted, so each core's rows are a contiguous slice of x). Rows are
host-padded so that each 128-segment block starts exactly at a 128-row tile
boundary -> every core runs the IDENTICAL instruction stream (pure SPMD),
only the data differs.

Two structural optimizations over the per-row mm1/mm2 formulation:
  1. x is transposed and cast to bf16 on the HOST. The device DMA reads
     xT [feat, rows] directly (2KB contiguous runs/partition), halving HBM
     traffic and eliminating all PE transposes of x.
  2. mm2 is deferred past the segment sum and folded into rho on the host:
       seg_sum(relu(x@W1+b1) @ W2 + b2) @ rho_w1
         = seg_sum(relu(x@W1+b1)) @ (W2 @ rho_w1) + counts * (b2 @ rho_w1)
     so the device only computes pseg = seg_sum(relu(x@W1)) [segs, 192] and
     a tiny per-block rho with E = W2 @ rho_w1 [192, 6]. The 400k-row mm2
     disappears entirely.

Per core device pipeline (fp8 mm1 / f32 psum accumulate / f32 rho):
  - DMA: xT chunks fp8e4m3 HBM -> SBUF [128 feat, 6 kc, 1024 rows]
    (x quantized to fp8 on host; W1 pre-scaled x16 into fp8 normal range,
    the 1/16 is folded exactly into E = W2 @ rho_w1 / 16)
  - mm1 per 128-row tile: out h1 [rows, 192] = sum_kc xT_kc.T @ W1_kc (PE,
    fp8 with FWL -- same stream rate as bf16 but half the DMA bytes)
  - ACT relu psum -> SBUF bf16 [rows, 192]
  - sel = is_equal(idx_local, iota) one-hot [128 rows, 128 segs] (DVE)
  - seg reduce: matmul(pseg += sel.T @ h1) accumulated in PSUM over ~tblk
    tiles -> pseg [128 segs, 192]; emission is pipelined 2 tiles behind
    mm1 so the relu/sel semaphores are already satisfied when PE gets there
  - rho (tiny, f32): transpose pseg, 2 matmuls + relu -> out [128] per block
"""

import sys

sys.path.insert(0, "/opt/trn_rl_repo")

import numpy as np
import ml_dtypes

N = 400000
B = 4096
DIN = 768
DHID = 192
NCORES = 8
SPC = B // NCORES  # segments per core = 512
SBLK = 128  # max segments per seg-block (psum accumulator width)
NBLKV = 5  # seg-blocks per core (variable seg ranges, packed on host)
P = 128
KC1 = DIN // P  # 6 k-chunks for mm1
CH = 1024  # rows per DMA chunk (8 row-tiles)

f32 = np.float32
bf16 = ml_dtypes.bfloat16
fp8 = ml_dtypes.float8_e4m3
W1_SCALE = 16.0  # pre-scale W1 into fp8 normal range; 1/16 folded into E


def _pack_blocks(counts, tpb):
    """Greedy per-core packing of 512 segments into NBLKV blocks of
    <= SBLK segments and <= tpb*128 rows each. Returns offs[c] (seg
    boundaries, len NBLKV+1) or None if tpb is too small."""
    offs = np.zeros((NCORES, NBLKV + 1), np.int64)
    for c in range(NCORES):
        seg = counts[c * SPC : (c + 1) * SPC]
        s = 0
        for b in range(NBLKV):
            offs[c, b] = s
            rows = 0
            while s < SPC and (s - offs[c, b]) < SBLK and rows + seg[s] <= tpb * P:
                rows += seg[s]
                s += 1
        if s != SPC:
            return None
        offs[c, NBLKV] = SPC
    return offs


def _prep(x, idx):
    """Host-side sharding. Returns per-core chunk-tiled fp8 shards + params."""
    if np.any(np.diff(idx) < 0):  # defensive: spec says idx is sorted
        order = np.argsort(idx, kind="stable")
        x, idx = x[order], idx[order]
    counts = np.bincount(idx, minlength=B)
    assert counts.sum() == x.shape[0]
    bounds = np.concatenate([[0], np.cumsum(counts)]).astype(np.int64)
    core_rows = bounds[SPC::SPC] - bounds[:-1:SPC]
    tpb = int(np.ceil(core_rows.max() / (NBLKV * P) / 8)) * 8  # %8 -> NP%1024
    offs = _pack_blocks(counts, tpb)
    while offs is None:
        tpb += 8
        offs = _pack_blocks(counts, tpb)
    NP = NBLKV * tpb * P
    nchunks = NP // CH
    xb = x.astype(fp8)
    # chunk-tiled layout: xts[c, ch, p, kc, n] = x_pad[ch*CH + n, kc*128 + p]
    # -> every chunk DMA is 128 descriptors of KC1*CH contiguous bytes
    xts = np.zeros((NCORES, NP, DIN), fp8)
    ixs = np.full((NCORES, NP), 1.0e9, f32)
    jmats = np.zeros((NCORES, NBLKV, SBLK), f32)
    for c in range(NCORES):
        for b in range(NBLKV):
            s0, s1 = c * SPC + offs[c, b], c * SPC + offs[c, b + 1]
            r0, r1 = int(bounds[s0]), int(bounds[s1])
            d0 = b * tpb * P
            xts[c, d0 : d0 + (r1 - r0)] = xb[r0:r1]
            ixs[c, d0 : d0 + (r1 - r0)] = (idx[r0:r1] - c * SPC).astype(f32)
            jmats[c, b] = offs[c, b] + np.arange(SBLK)
    xts = np.ascontiguousarray(
        xts.reshape(NCORES, nchunks, CH, KC1, P).transpose(0, 1, 4, 3, 2)
    )
    jmats = np.ascontiguousarray(np.broadcast_to(jmats[:, None], (NCORES, P, NBLKV, SBLK)))
    # pre-arrange idx so each partition's DMA read is contiguous:
    # ixs_arr[c, ch, p, n] = ixs[c, ch*CH + n*P + p]
    ixs_arr = np.ascontiguousarray(
        ixs.reshape(NCORES, nchunks, CH // P, P).transpose(0, 1, 3, 2)
    )
    return xts, ixs_arr, tpb, counts, jmats, offs


def _build(tpb, phi_w1, phi_b1, phi_w2, phi_b2, rho_w1, rho_b1, rho_w2, rho_b2):
    import concourse.bacc as bacc
    import concourse.mybir as mybir
    import concourse.tile as tile

    BF = mybir.dt.bfloat16
    F32 = mybir.dt.float32
    FP8 = mybir.dt.float8e4
    Relu = mybir.ActivationFunctionType.Relu
    Copy = mybir.ActivationFunctionType.Copy

    has_b1 = bool(np.any(phi_b1 != 0))
    has_b2 = bool(np.any(phi_b2 != 0))
    has_rb1 = bool(np.any(rho_b1 != 0))
    has_rb2 = bool(np.any(rho_b2 != 0))

    # ---- packed constants (inlined into the NEFF) ----
    # w1k[p, kc, h] = W1[kc*128 + p, h] * 16 (fp8 normal range)
    w1k = np.ascontiguousarray(
        (phi_w1 * W1_SCALE).reshape(KC1, P, DHID).transpose(1, 0, 2)
    ).astype(fp8)
    # E = W2 @ rho_w1 / 16  [192, 6] (mm2 + the 1/16 folded into rho)
    E = (phi_w2.astype(np.float64) @ rho_w1.astype(np.float64)).astype(f32) / W1_SCALE
    rw1k = np.ascontiguousarray(E.reshape(2, 96, 6).transpose(1, 0, 2)).astype(f32)
    rw2k = np.ascontiguousarray(rho_w2).astype(f32)  # [6, 1]
    idn32 = np.eye(P, dtype=f32)
    rb1k = np.ascontiguousarray(rho_b1.reshape(6, 1)).astype(f32)
    rb2k = np.ascontiguousarray(rho_b2.reshape(1, 1)).astype(f32)
    onesk = np.ones((1, P), bf16)
    b1k = np.ascontiguousarray(phi_b1.reshape(1, DHID) * W1_SCALE).astype(bf16)
    # beta = b2 @ rho_w1 [6]; pseg misses counts*b2, corrected in rho preact
    beta = (phi_b2.astype(np.float64) @ rho_w1.astype(np.float64)).astype(f32)
    betak = np.ascontiguousarray(beta.reshape(1, 6)).astype(f32)

    NP = NBLKV * tpb * P
    nchunks = NP // CH

    nc = bacc.Bacc(None, target_bir_lowering=False)
    xt_in = nc.dram_tensor("xt_shard", [NP // CH, P, KC1, CH], FP8, kind="ExternalInput")
    ix_in = nc.dram_tensor("idxlf", [nchunks, P, CH // P], F32, kind="ExternalInput")
    jm_in = nc.dram_tensor("jmat", [P, NBLKV, SBLK], F32, kind="ExternalInput")
    cnt_in = (
        nc.dram_tensor("cnts", [1, NBLKV, SBLK], F32, kind="ExternalInput")
        if has_b2
        else None
    )
    out_d = nc.dram_tensor("out_shard", [NBLKV * SBLK], F32, kind="ExternalOutput")

    w1d = nc.inline_tensor(w1k, "w1k")
    rw1d = nc.inline_tensor(rw1k, "rw1k")
    rw2d = nc.inline_tensor(rw2k, "rw2k")
    idn32d = nc.inline_tensor(idn32, "idn32")
    rb1d = nc.inline_tensor(rb1k, "rb1k") if has_rb1 else None
    rb2d = nc.inline_tensor(rb2k, "rb2k") if has_rb2 else None
    onesd = nc.inline_tensor(onesk, "onesk") if has_b1 else None
    b1d = nc.inline_tensor(b1k, "b1k") if has_b1 else None
    betad = nc.inline_tensor(betak, "betak") if has_b2 else None

    with tile.TileContext(nc) as tc:
        with (
            tc.tile_pool(name="consts", bufs=1) as cpool,
            tc.tile_pool(name="xb", bufs=6) as xpool,
            tc.tile_pool(name="ixb", bufs=6) as ixpool,
            tc.tile_pool(name="h1b", bufs=6) as h1pool,
            tc.tile_pool(name="selb", bufs=6) as selpool,
            tc.tile_pool(name="rho", bufs=2) as rhopool,
            tc.tile_pool(name="ph1", bufs=4, space="PSUM") as ph1,
            tc.tile_pool(name="pxt", bufs=2, space="PSUM") as pxt,
            tc.tile_pool(name="pseg", bufs=2, space="PSUM") as pseg,
        ):
            # w1s is the only const mm1 needs -- load it first so PE can
            # start as soon as the first x piece lands; everything else
            # (sel iota, rho consts) is loaded after the first x DMAs.
            w1s = cpool.tile_from(w1d[:], name="w1s")
            late = {}

            def load_late_consts():
                late["js"] = cpool.tile_from(jm_in[:], name="js")
                late["rw1s"] = cpool.tile_from(rw1d[:], name="rw1s")
                late["rw2s"] = cpool.tile_from(rw2d[:], name="rw2s")
                late["idn32s"] = cpool.tile_from(idn32d[:], name="idn32s")
                late["rb1s"] = cpool.tile_from(rb1d[:], name="rb1s") if has_rb1 else None
                late["rb2s"] = cpool.tile_from(rb2d[:], name="rb2s") if has_rb2 else None
                late["oness"] = cpool.tile_from(onesd[:], name="oness") if has_b1 else None
                late["b1s"] = cpool.tile_from(b1d[:], name="b1s") if has_b1 else None
                late["betas"] = cpool.tile_from(betad[:], name="betas") if has_b2 else None
                late["cnts"] = cpool.tile_from(cnt_in[:], name="cnts") if has_b2 else None

            pseg_tiles = {}
            pend_seg = []  # [(selb, h1s, blk, pos)] 2-deep PE emission queue
            pend_rho = None  # (blk, pseg_tile)

            def emit_seg(selb_t, h1s_t, blk, pos):
                nc.tensor.matmul(
                    out=pseg_tiles[blk][:],
                    lhsT=selb_t[:],
                    rhs=h1s_t[:],
                    start=(pos == 0),
                    stop=(pos == tpb - 1),
                )
                if pos == tpb - 1:
                    return (blk, pseg_tiles.pop(blk))
                return None

            def emit_rho(blk, pseg_t):
                # pseg [128 segs, 192] f32 psum -> out[blk*128:(blk+1)*128]
                xsb = rhopool.tile([P, DHID], F32, tag="xsb")
                nc.scalar.copy(out=xsb[:], in_=pseg_t[:])
                pxsT = pxt.tile([96, 2, P], F32, tag="xt")
                for m2 in range(2):
                    nc.tensor.transpose(
                        out=pxsT[:, m2, :],
                        in_=xsb[:, m2 * 96 : (m2 + 1) * 96],
                        identity=late["idn32s"][:],
                    )
                xsTb = rhopool.tile([96, 2, P], F32, tag="xsTb")
                nc.vector.tensor_copy(out=xsTb[:], in_=pxsT[:])
                prt = pxt.tile([6, P], F32, tag="xt")
                for m2 in range(2):
                    nc.tensor.matmul(
                        out=prt[:],
                        lhsT=late["rw1s"][:, m2, :],
                        rhs=xsTb[:, m2, :],
                        start=(m2 == 0),
                        stop=(m2 == 1 and not has_b2),
                    )
                if has_b2:
                    nc.tensor.matmul(
                        out=prt[:], lhsT=late["betas"][:], rhs=late["cnts"][:, blk, :],
                        start=False, stop=True,
                    )
                rtb = rhopool.tile([6, P], F32, tag="rtb")
                if has_rb1:
                    nc.scalar.activation(
                        out=rtb[:], in_=prt[:], func=Relu, bias=late["rb1s"][:]
                    )
                else:
                    nc.scalar.activation(out=rtb[:], in_=prt[:], func=Relu)
                pot = pxt.tile([1, P], F32, tag="xt")
                nc.tensor.matmul(out=pot[:], lhsT=late["rw2s"][:], rhs=rtb[:], start=True, stop=True)
                ob = rhopool.tile([1, P], F32, tag="ob")
                if has_rb2:
                    nc.scalar.activation(out=ob[:], in_=pot[:], func=Copy, bias=late["rb2s"][:])
                else:
                    nc.scalar.copy(out=ob[:], in_=pot[:])
                nc.sync.dma_start(out=out_d[blk * SBLK : (blk + 1) * SBLK], in_=ob[:])

            state = {"pend_seg": pend_seg, "pend_rho": pend_rho}

            def do_tile(t, i, xt_at, ixb, ph1_pre=None):
                blk = t // tpb
                pos = t % tpb
                # --- mm1: h1 [rows, 192] = x_tile @ W1 ---
                if ph1_pre is not None:
                    ph1t, kc0 = ph1_pre
                else:
                    ph1t = ph1.tile([P, DHID], F32, tag="h1", name=f"ph1_{t}")
                    kc0 = 0
                for kc in range(kc0, KC1):
                    nc.tensor.matmul(
                        out=ph1t[:],
                        lhsT=xt_at(i, kc),
                        rhs=w1s[:, kc, :],
                        start=(kc == 0),
                        stop=(kc == KC1 - 1 and not has_b1),
                    )
                if has_b1:
                    nc.tensor.matmul(
                        out=ph1t[:], lhsT=late["oness"][:], rhs=late["b1s"][:],
                        start=False, stop=True,
                    )
                # pipelined tails from 2 tiles ago (relu/sel sems are
                # already satisfied by the time PE reaches the seg matmul)
                if state["pend_rho"] is not None:
                    emit_rho(*state["pend_rho"])
                    state["pend_rho"] = None
                if len(state["pend_seg"]) >= 2:
                    state["pend_rho"] = emit_seg(*state["pend_seg"].pop(0))
                # --- relu + one-hot sel for this tile ---
                h1s = h1pool.tile([P, DHID], BF, tag="h1s", name=f"h1s_{t}")
                nc.scalar.activation(out=h1s[:], in_=ph1t[:], func=Relu)
                selb = selpool.tile([P, P], BF, tag="selb", name=f"sel_{t}")
                nc.vector.tensor_tensor(
                    out=selb[:],
                    in0=ixb[:, i : i + 1].to_broadcast([P, P]),
                    in1=late["js"][:, blk, :],
                    op=mybir.AluOpType.is_equal,
                )
                if pos == 0:
                    pseg_tiles[blk] = pseg.tile(
                        [P, DHID], F32, tag="seg", name=f"pseg_{blk}"
                    )
                state["pend_seg"].append((selb, h1s, blk, pos))

            # chunk 0: two kc-group pieces (128 descriptors of 3KB each --
            # cheap to generate AND transfer). mm1 for tiles 0-3 is split
            # into two passes (kc0-2 then kc3-5) so PE starts on piece A
            # while piece B is still in flight.
            xq_a = xpool.tile([P, 3, CH], FP8, tag="xb0a", name="xb0a", bufs=1)
            nc.gpsimd.dma_start(out=xq_a[:], in_=xt_in[0, :, 0:3, :])
            ixb0 = ixpool.tile([P, CH // P], F32, tag="ixb", name="ixb0")
            nc.sync.dma_start(out=ixb0[:], in_=ix_in[0])

            xq_b = xpool.tile([P, 3, CH], FP8, tag="xb0b", name="xb0b", bufs=1)
            nc.gpsimd.dma_start(out=xq_b[:], in_=xt_in[0, :, 3:6, :])
            load_late_consts()

            def xt_at0(i, kc):
                tt = xq_a if kc < 3 else xq_b
                return tt[:, kc % 3, i * P : (i + 1) * P]

            NWARM = 4  # tiles 0..3: kc0-2 first (piece A), then kc3-5
            ph1_warm = []
            for i in range(NWARM):
                ph1t = ph1.tile([P, DHID], F32, tag="h1", name=f"ph1_{i}")
                for kc in range(3):
                    nc.tensor.matmul(
                        out=ph1t[:],
                        lhsT=xt_at0(i, kc),
                        rhs=w1s[:, kc, :],
                        start=(kc == 0),
                        stop=False,
                    )
                ph1_warm.append(ph1t)
            for i in range(NWARM):
                do_tile(i, i, xt_at0, ixb0, ph1_pre=(ph1_warm[i], 3))
            for i in range(NWARM, CH // P):
                do_tile(i, i, xt_at0, ixb0)

            for ch in range(1, nchunks):
                xtb = xpool.tile([P, KC1, CH], FP8, tag="xtb", name=f"xtb_{ch}")
                nc.gpsimd.dma_start(out=xtb[:], in_=xt_in[ch])

                def xt_at(i, kc, _xtb=xtb):
                    return _xtb[:, kc, i * P : (i + 1) * P]

                ixb = ixpool.tile([P, CH // P], F32, tag="ixb", name=f"ixb_{ch}")
                nc.sync.dma_start(out=ixb[:], in_=ix_in[ch])
                for i in range(CH // P):
                    do_tile(ch * (CH // P) + i, i, xt_at, ixb)
            pend_seg = state["pend_seg"]
            pend_rho = state["pend_rho"]
            # drain
            while pend_seg:
                if pend_rho is not None:
                    emit_rho(*pend_rho)
                    pend_rho = None
                pend_rho = emit_seg(*pend_seg.pop(0))
            if pend_rho is not None:
                emit_rho(*pend_rho)
                pend_rho = None

    nc.compile()
    return nc


_CACHE = {}


def _get_nc(tpb, weights):
    key = (tpb, tuple(hash(w.tobytes()) for w in weights))
    if key not in _CACHE:
        _CACHE[key] = _build(tpb, *weights)
    return _CACHE[key]


def _run(inputs, trace=False):
    from concourse.bass_utils import run_bass_kernel_spmd

    inp = {k: np.asarray(v) for k, v in inputs.items()}
    x = inp["x"].astype(f32, copy=False)
    idx = inp["idx"].astype(np.int32, copy=False)
    weights = tuple(
        inp[k].astype(f32, copy=False)
        for k in ("phi_w1", "phi_b1", "phi_w2", "phi_b2", "rho_w1", "rho_b1", "rho_w2", "rho_b2")
    )
    xts, ixs, tpb, counts, jmats, offs = _prep(x, idx)
    nc = _get_nc(tpb, weights)
    has_b2 = bool(np.any(weights[3] != 0))
    in_maps = []
    for c in range(NCORES):
        m = {"xt_shard": xts[c], "idxlf": ixs[c], "jmat": jmats[c]}
        if has_b2:
            cp = np.zeros((1, NBLKV, SBLK), f32)
            for b in range(NBLKV):
                ns = int(offs[c, b + 1] - offs[c, b])
                cp[0, b, :ns] = counts[
                    c * SPC + offs[c, b] : c * SPC + offs[c, b + 1]
                ]
            m["cnts"] = cp
        in_maps.append(m)
    res = run_bass_kernel_spmd(nc, in_maps, core_ids=list(range(NCORES)), trace=trace)
    # scatter the per-block 128-slot outputs back to segment order
    out = np.empty(B, f32)
    for c in range(NCORES):
        r = np.asarray(res.results[c]["out_shard"], f32)
        for b in range(NBLKV):
            ns = int(offs[c, b + 1] - offs[c, b])
            out[c * SPC + offs[c, b] : c * SPC + offs[c, b + 1]] = r[
                b * SBLK : b * SBLK + ns
            ]
    out = out.reshape(B, 1)
    return out, res


def kernel(**inputs) -> np.ndarray:
    return _run(inputs, trace=False)[0]


if __name__ == "__main__":
    # quick self-test against numpy
    rng = np.random.default_rng(0)
    x = rng.standard_normal((N, DIN)).astype(f32)
    idx = np.sort(rng.integers(0, B, N).astype(np.int32))
    w1 = (rng.standard_normal((DIN, DHID)) / np.sqrt(DIN)).astype(f32)
    w2 = (rng.standard_normal((DHID, DHID)) / np.sqrt(DHID)).astype(f32)
    r1 = (rng.standard_normal((DHID, 6)) / np.sqrt(DHID)).astype(f32)
    r2 = (rng.standard_normal((6, 1)) / np.sqrt(6)).astype(f32)
    inputs = dict(
        x=x, idx=idx,
        phi_w1=w1, phi_b1=np.zeros(DHID, f32), phi_w2=w2, phi_b2=np.zeros(DHID, f32),
        rho_w1=r1, rho_b1=np.zeros(6, f32), rho_w2=r2, rho_b2=np.zeros(1, f32),
    )
    out = kernel(**inputs)
    h = np.maximum(x @ w1, 0.0) @ w2
    xsum = np.zeros((B, DHID), f32)
    np.add.at(xsum, idx, h)
    exp = np.maximum(xsum @ r1, 0.0) @ r2
    rel = np.linalg.norm(out - exp) / np.linalg.norm(exp)
    print("self-test rel err:", rel)
